# revision 9
# baseline (speedup 1.0000x reference)
"""Trainium2 Bass kernel for nn_DigitConvolutionalModel.

Model: x(B,784) -> reshape 28x28 -> 3x3 valid cross-correlation (kernel is an
input) -> flatten 676 -> Linear(676,128)+ReLU -> Linear(128,10).

Strategy:
  * Fold the 3x3 conv into the first linear layer on the host: the conv is a
    linear map, so h = relu(x @ W1eff.T + b1) with W1eff (128, 784) built by
    scattering conv_w-weighted copies of w1 onto the 28x28 grid. The device
    kernel is then a plain 2-layer MLP over 784 features.
  * Pure data parallelism: batch 65536 split as 8192 rows per NeuronCore,
    weights replicated.
  * The PE streams 1 column/cycle regardless of operand dtype (fp8 gets no
    2x here: DoubleRow perf mode needs e4m3/e5m2 on BOTH operands, whose
    3-bit mantissa measures 2.5e-2 against the 2e-2 gate). Layer-1 is 7
    chunk-matmuls + 1 layer-2 matmul = 8 cycles/sample = ~27.3us/core at
    2.4GHz. That is the compute floor.
  * The 16 SDMA engines run ~15-19 GB/s each (~245 GB/s aggregate) at any
    descriptor size >= 256B, so DMA time is proportional to bytes: fp16 x
    would be 12.8MB (~48us, DMA-bound), full e3m4 is 6.4MB (~26us) which
    rides just under the PE floor. e3m4 keeps 4 mantissa bits; all-e3m4
    measures ~1.65e-2 on HW against the 2e-2 gate.
  * A single HWDGE queue caps at ~140 GB/s, so x blocks alternate between
    the two HWDGE rings (sync + ACT), each block shipped whole via one
    queue as 112 fully contiguous per-partition runs of 7*xb bytes.
    Queue assignment is greedy on cumulative bytes (w1 rides sync, w2p
    rides ACT, both ahead of the bulk so the first matmul isn't blocked).
  * The HAM clock gate holds the PE at 1.2 GHz until it has seen ~5us of
    sustained activity, and a PE stall resets the ramp. So: dummy warm-up
    matmuls bridge the window between the framework preamble and x block
    0's arrival (block 0 is small so it lands ASAP), and the block
    schedule keeps DMA slightly ahead of the PE throughout. The last
    block is small so the serialized relu->L2->bias->store tail is short.
  * Engine layout: DVE does relu+bias epilogues, gpsimd/SWDGE stores
    per-block outputs so stores never head-of-line-block the x stream;
    the final store goes on the (by then idle) sync ring.
"""

from contextlib import ExitStack

import numpy as np

B = 65536
H = W = 28
K = 3
CH = CW = 26
FEAT = H * W          # 784
HID = 128
OUT = 10
NCORES = 8
BC = B // NCORES      # 8192 rows per core

KC = 112              # contraction-chunk partition size
KCH = 7               # chunks: 7 * 112 = 784
NT = 512              # max batch rows per compute tile (one PSUM bank fp32)
NWARM = 12            # 512-col dummy matmuls bridging preamble -> block 0
                      # (block 0 can't land before ~13us: first-DMA spin-up
                      # is ~4-5us; a post-warmup PE gap costs a 10us HALF-clock
                      # window, so over-provision)
X8SCALE = 2.0         # x is quantized as e3m4(2*x); 0.5 folded into w1

# variant "mixN": N chunks of x shipped as fp8-e3m4, 7-N as fp16.
# "f16" == mix0 (safe, ~5e-4 err), "f8" == mix7 (~1.65e-2 err).
VARIANT = "f8"

_NC_CACHE = {}


def _n8(variant):
    if variant == "f16":
        return 0
    if variant == "f8":
        return KCH
    if variant.startswith("mix"):
        n = int(variant[3:])
        assert 0 <= n <= KCH
        return n
    raise ValueError(variant)


def _blocks(bc):
    if bc == 8192:
        # small blocks while the PE is cold (1.2GHz), bigger once hot; the
        # tiny first block lands ASAP so the real stream starts early, the
        # tiny last block keeps the serialized epilogue tail short
        blocks = [256, 256, 512, 512, 512, 768, 768, 1280, 1280, 1280, 384, 256, 128]
    else:
        step = min(1024, bc)
        blocks = [min(step, bc - o) for o in range(0, bc, step)]
    assert sum(blocks) == bc
    return blocks


def _queues(blocks, wid):
    """Greedy queue assignment (0=sync, 1=ACT) balancing cumulative bytes.
    Sync starts with w1 (~200KB) on it, ACT with w2p (~8KB)."""
    load = [KC * KCH * HID * 2, 32 * HID * 2]
    qs = []
    for bi, xb in enumerate(blocks):
        if bi < 2:
            q = bi  # b0 leads the sync ring, b1 leads the ACT ring
        else:
            q = 0 if load[0] <= load[1] else 1
        qs.append(q)
        load[q] += KC * wid * xb
    return qs


def _tiles(xb):
    out, t0 = [], 0
    while t0 < xb:
        nt = min(NT, xb - t0)
        out.append((t0, nt))
        t0 += nt
    return out


def _build_nc(bc, variant):
    from concourse import bacc
    import concourse.mybir as mybir
    import concourse.tile as tile

    f32 = mybir.dt.float32
    f16 = mybir.dt.float16
    f8 = mybir.dt.float8e3
    u8 = mybir.dt.uint8
    n8 = _n8(variant)
    # chunk c dtype: first n8 chunks fp8, rest fp16
    csz = [1 if c < n8 else 2 for c in range(KCH)]
    cdt = [f8 if c < n8 else f16 for c in range(KCH)]
    coff = [sum(csz[:c]) for c in range(KCH)]  # byte offset factor per chunk
    wid = sum(csz)                             # bytes per column, all chunks
    blocks = _blocks(bc)
    queues = _queues(blocks, wid)

    nc = bacc.Bacc(
        "TRN2",
        target_bir_lowering=False,
        debug=False,
        enable_asserts=False,
        num_devices=NCORES,
    )
    # Per-queue dram tensor; per partition, that queue's blocks in order,
    # each block packed [chunk, col] so a block load is one contiguous run
    # of wid*xb bytes per partition.
    # NOTE: partition dim must stay 112 — a 113-partition DMA defeats the
    # HWDGE descriptor spray and funnels the whole queue through ONE SDMA
    # engine (measured: 8x bandwidth collapse).
    qbytes = [sum(wid * xb for xb, q in zip(blocks, queues) if q == i) for i in (0, 1)]
    xQ0 = nc.dram_tensor("xQ0", [KC, qbytes[0]], u8, kind="ExternalInput").ap()
    xQ1 = nc.dram_tensor("xQ1", [KC, qbytes[1]], u8, kind="ExternalInput").ap()
    w1t = nc.dram_tensor("w1t", [KC, KCH, HID], f16, kind="ExternalInput").ap()
    # w2, b2 and b1 packed as one [32, HID] tensor (rows 0-9 = w2, row 10 =
    # b2, row 11 = b1): 32 fat descriptors instead of 266 4-20B ones (each
    # tiny descriptor is a full HBM round trip), transposed on the DVE
    w2p = nc.dram_tensor("w2p", [32, HID], f16, kind="ExternalInput").ap()
    outT = nc.dram_tensor("outT", [OUT, bc], f32, kind="ExternalOutput").ap()

    with ExitStack() as ctx:
        tc = ctx.enter_context(tile.TileContext(nc))
        wpool = ctx.enter_context(tc.tile_pool(name="w", bufs=1))
        xpool = ctx.enter_context(tc.tile_pool(name="x", bufs=3))
        hpool = ctx.enter_context(tc.tile_pool(name="h", bufs=3))
        opool = ctx.enter_context(tc.tile_pool(name="o", bufs=4))
        p1pool = ctx.enter_context(tc.tile_pool(name="p1", bufs=5, space="PSUM"))
        p2pool = ctx.enter_context(tc.tile_pool(name="p2", bufs=3, space="PSUM"))

        qeng = [nc.sync, nc.scalar]
        qoff = [0, 0]      # running byte offset within each queue tensor
        qt = [xQ0, xQ1]
        xs = [None] * len(blocks)
        xoff = [None] * len(blocks)

        def issue_x(bi):
            xb = blocks[bi]
            q = queues[bi]
            xs[bi] = xpool.tile(
                [KC, wid * xb], u8, tag=f"xs_{xb}", name=f"xs_{bi}"
            )
            qeng[q].dma_start(xs[bi][:], qt[q][:, qoff[q] : qoff[q] + wid * xb])
            qoff[q] += wid * xb

        # w1 chunk 0 is its own small DMA so the very first matmul doesn't
        # wait on all 7 chunks; x block 0 goes right behind it, then the
        # w1 bulk. On the ACT ring w2p (tiny) leads its first x block.
        w1s = wpool.tile([KC, KCH, HID], f16)
        w2ps = wpool.tile([32, HID], f16)
        issue_x(0)
        issue_x(1)
        nc.sync.dma_start(w1s[:, 0:1, :], w1t[:, 0:1, :])
        nc.scalar.dma_start(w2ps[:], w2p[:])
        nc.sync.dma_start(w1s[:, 1:KCH, :], w1t[:, 1:KCH, :])

        w2sT = wpool.tile([HID, 32], f16)
        for j in range(HID // 32):
            # DVE transpose flips one 32x32 block; stitch the full transpose
            nc.vector.transpose(
                w2sT[32 * j : 32 * (j + 1), 0:32], w2ps[0:32, 32 * j : 32 * (j + 1)]
            )
        w2l = w2sT[:, 0:OUT]        # layer-2 lhsT [128, 10]
        bvec = wpool.tile([HID, 2], f32)  # tensor_scalar wants f32 scalars
        nc.vector.tensor_copy(bvec[:], w2sT[:, OUT : OUT + 2])
        b2s = bvec[0:OUT, 0:1]      # b2 as per-partition scalar [10, 1]
        b1s = bvec[:, 1:2]          # b1 as per-partition scalar [128, 1]

        # PE warm-up: the HAM clock gate holds the PE at 1.2 GHz until it
        # has seen ~5us of sustained activity, and a PE stall resets the
        # ramp. Burn dummy matmuls on scratch during block 0's DMA so the
        # PE never idles between the framework preamble and the real
        # stream. Garbage operands are fine: start=True overwrites PSUM,
        # and the first real matmul overwrites it again.
        warmW = wpool.tile([KC, HID], f16)
        warmM = wpool.tile([KC, NT], f16)
        nc.gpsimd.memset(warmW[:], 0.0)   # two engines so the first warm
        nc.vector.memset(warmM[:], 0.0)   # LDWEIGHTS isn't gated on one big memset
        pwarm = p1pool.tile([HID, NT], f32, tag="p1", name="p1_warm")
        for i in range(NWARM):
            nc.tensor.matmul(
                pwarm[:], warmW[:], warmM[:],
                start=True, stop=True, skip_group_check=True,
            )

        # global tile list: (blk_idx, xb, t0, nt, first_of_block)
        gtiles = []
        for bi, xb in enumerate(blocks):
            for ti, (t0, nt) in enumerate(_tiles(xb)):
                gtiles.append((bi, xb, t0, nt, ti == 0))

        os_ = [None] * len(blocks)
        done_tiles = [0] * len(blocks)  # epilogues emitted per block
        ntiles_of = [len(_tiles(xb)) for xb in blocks]
        # software pipeline: the L2 matmul of tile j-2 is emitted while the
        # PE chews on tile j's layer-1, so the DVE relu (emitted right after
        # tile j-2's layer-1) has two full tiles of slack before the PE
        # needs its output
        pend = []  # [(p1 tile, bi, t0, nt), ...]

        add = mybir.AluOpType.add
        mx = mybir.AluOpType.max

        def flush(k):
            # drain k pending tiles as a batch: both relus first (DVE),
            # then both L2 matmuls back-to-back (one w2l->w1c0 PE weight
            # transition per batch instead of per tile), then the bias
            # epilogues on the otherwise-idle gpsimd engine
            batch = [pend.pop(0) for _ in range(k)]
            hss = []
            for p1, bi, t0, nt in batch:
                hs = hpool.tile([HID, nt], f16, tag="hs", name=f"hs_{bi}_{t0}")
                nc.vector.tensor_scalar(hs[:], p1[:], b1s, 0.0, add, mx)
                hss.append(hs)
            p2s = []
            for (p1, bi, t0, nt), hs in zip(batch, hss):
                p2 = p2pool.tile([OUT, nt], f32, tag="p2", name=f"p2_{bi}_{t0}")
                nc.tensor.matmul(p2[:], w2l, hs[:], start=True, stop=True)
                p2s.append(p2)
            for (p1, bi, t0, nt), p2 in zip(batch, p2s):
                # gpsimd can't read PSUM; the ACT ring only issues DMAs and
                # has plenty of slack, so the bias epilogue rides there
                nc.scalar.activation(
                    os_[bi][:, t0 : t0 + nt], p2[:],
                    mybir.ActivationFunctionType.Identity, bias=b2s,
                )
                done_tiles[bi] += 1
                # store a fully finished block via SWDGE (never blocks x loads)
                if done_tiles[bi] == ntiles_of[bi] and bi < len(blocks) - 1:
                    boff = sum(blocks[:bi])
                    nc.gpsimd.dma_start(outT[:, boff : boff + blocks[bi]], os_[bi][:])
                    os_[bi] = None

        for bi, xb, t0, nt, first in gtiles:
            if first:
                # per-size tags: mixing tile sizes in one tag ring makes the
                # pool heap overlap buffers and serialize on stale readers
                if bi > 1:
                    issue_x(bi)
                os_[bi] = opool.tile([OUT, xb], f32, tag=f"os_{xb}", name=f"os_{bi}")

            # emit the epilogues for the tiles the DVE should do next,
            # BEFORE this tile's matmuls, so the PE keeps 2+ tiles of slack
            if len(pend) >= 4:
                flush(2)

            p1 = p1pool.tile([HID, nt], f32, tag="p1", name=f"p1_{bi}_{t0}")
            for c in range(KCH):
                base = coff[c] * xb
                if csz[c] == 1:
                    rhs = xs[bi][:, base + t0 : base + t0 + nt].bitcast(f8)
                else:
                    rhs = xs[bi][:, base + 2 * t0 : base + 2 * (t0 + nt)].bitcast(f16)
                nc.tensor.matmul(
                    p1[:],
                    w1s[:, c, :],
                    rhs,
                    start=(c == 0),
                    stop=(c == KCH - 1),
                )
            pend.append((p1, bi, t0, nt))

        while pend:
            flush(min(2, len(pend)))
        off = 0
        for bi, xb in enumerate(blocks):
            if os_[bi] is not None:
                # the final stores go on the (now idle) sync ring: HWDGE
                # latency on the critical tail
                nc.sync.dma_start(outT[:, off : off + xb], os_[bi][:])
                os_[bi] = None
            off += xb

    nc.compile()
    return nc


def get_nc(bc=BC, variant=VARIANT):
    key = (bc, variant)
    if key not in _NC_CACHE:
        _NC_CACHE[key] = _build_nc(bc, variant)
    return _NC_CACHE[key]


def _host_prep(x, conv_w, w1, b1, w2, b2, variant):
    """Fold conv into layer-1 weights and lay out per-core device inputs."""
    import ml_dtypes

    n8 = _n8(variant)
    csz = [1 if c < n8 else 2 for c in range(KCH)]
    wid = sum(csz)

    x = np.asarray(x, dtype=np.float32)
    conv_w = np.asarray(conv_w, dtype=np.float32)
    w1 = np.asarray(w1, dtype=np.float32)
    b1 = np.asarray(b1, dtype=np.float32)
    w2 = np.asarray(w2, dtype=np.float32)
    b2 = np.asarray(b2, dtype=np.float32)

    w1_img = w1.reshape(HID, CH, CW)
    w1eff = np.zeros((HID, H, W), dtype=np.float32)
    for di in range(K):
        for dj in range(K):
            w1eff[:, di : di + CH, dj : dj + CW] += conv_w[di, dj] * w1_img
    w1eff = w1eff.reshape(HID, FEAT)

    # [784,128] -> [7,112,128] -> [112,7,128]; fp8 chunks carry the folded
    # 1/X8SCALE so the device dequant is free
    w1full = w1eff.T.reshape(KCH, KC, HID).copy()
    w1full[:n8] *= 1.0 / X8SCALE
    w1t_host = np.ascontiguousarray(w1full.transpose(1, 0, 2)).astype(np.float16)
    # rows 0-9: w2; row 10: b2; row 11: b1 (device reads them back out of
    # the DVE-transposed tile as per-partition scalar columns)
    w2p_host = np.zeros((32, HID), dtype=np.float16)
    w2p_host[:OUT] = w2.astype(np.float16)
    w2p_host[OUT, :OUT] = b2.astype(np.float16)
    w2p_host[OUT + 1] = b1.astype(np.float16)

    blocks = _blocks(BC)
    queues = _queues(blocks, wid)
    in_maps = []
    for c in range(NCORES):
        shardT = x[c * BC : (c + 1) * BC].T  # [784, BC] view
        xr = np.ascontiguousarray(shardT).reshape(KCH, KC, BC)
        # per-chunk byte rows [KC, csz*BC]
        rows = []
        for ci in range(KCH):
            if csz[ci] == 1:
                q = (xr[ci] * X8SCALE).astype(ml_dtypes.float8_e3m4)
                rows.append(q.view(np.uint8))
            else:
                q = np.ascontiguousarray(xr[ci].astype(np.float16))
                rows.append(q.view(np.uint8).reshape(KC, 2 * BC))
        qhost = [np.empty((KC, 0), np.uint8), np.empty((KC, 0), np.uint8)]
        parts = [[], []]
        off = 0
        for xb, qi in zip(blocks, queues):
            for ci in range(KCH):
                w = csz[ci]
                parts[qi].append(rows[ci][:, w * off : w * (off + xb)])
            off += xb
        qhost = [
            np.concatenate(p, axis=1) if p else np.empty((KC, 0), np.uint8)
            for p in parts
        ]
        im = {
            "w1t": w1t_host,
            "w2p": w2p_host,
            "xQ0": np.ascontiguousarray(qhost[0]),
            "xQ1": np.ascontiguousarray(qhost[1]),
        }
        in_maps.append(im)
    return in_maps


def run(x, conv_w, w1, b1, w2, b2, trace=False, variant=VARIANT):
    from concourse.bass_utils import run_bass_kernel_spmd

    in_maps = _host_prep(x, conv_w, w1, b1, w2, b2, variant)
    nc = get_nc(BC, variant)
    res = run_bass_kernel_spmd(nc, in_maps, list(range(NCORES)), trace=trace)
    outT = np.concatenate([r["outT"] for r in res.results], axis=1)  # [10, B]
    return np.ascontiguousarray(outT.T), res


def kernel(x, conv_w, w1, b1, w2, b2):
    out, _ = run(x, conv_w, w1, b1, w2, b2)
    return out


# revision 10
# speedup vs baseline: 1.0944x; 1.0944x over previous
"""Trainium2 Bass kernel for nn_DigitConvolutionalModel.

Model: x(B,784) -> reshape 28x28 -> 3x3 valid cross-correlation (kernel is an
input) -> flatten 676 -> Linear(676,128)+ReLU -> Linear(128,10).

Strategy:
  * Fold the 3x3 conv into the first linear layer on the host: the conv is a
    linear map, so h = relu(x @ W1eff.T + b1) with W1eff (128, 784) built by
    scattering conv_w-weighted copies of w1 onto the 28x28 grid. The device
    kernel is then a plain 2-layer MLP over 784 features.
  * Pure data parallelism: batch 65536 split as 8192 rows per NeuronCore,
    weights replicated.
  * The PE streams 1 column/cycle regardless of operand dtype (fp8 gets no
    2x here: DoubleRow perf mode needs e4m3/e5m2 on BOTH operands, whose
    3-bit mantissa measures 2.5e-2 against the 2e-2 gate). Layer-1 is 7
    chunk-matmuls + 1 layer-2 matmul = 8 cycles/sample = ~27.3us/core at
    2.4GHz. That is the compute floor.
  * The 16 SDMA engines run ~15-19 GB/s each (~245 GB/s aggregate) at any
    descriptor size >= 256B, so DMA time is proportional to bytes: fp16 x
    would be 12.8MB (~48us, DMA-bound), full e3m4 is 6.4MB (~26us) which
    rides just under the PE floor. e3m4 keeps 4 mantissa bits; all-e3m4
    measures ~1.65e-2 on HW against the 2e-2 gate.
  * A single HWDGE queue caps at ~140 GB/s, so x blocks alternate between
    the two HWDGE rings (sync + ACT), each block shipped whole via one
    queue as 112 fully contiguous per-partition runs of 7*xb bytes.
    Queue assignment is greedy on cumulative bytes (w1 rides sync, w2p
    rides ACT, both ahead of the bulk so the first matmul isn't blocked).
  * The HAM clock gate holds the PE at 1.2 GHz until it has seen ~5us of
    sustained activity, and a PE stall resets the ramp. So: dummy warm-up
    matmuls bridge the window between the framework preamble and x block
    0's arrival (block 0 is small so it lands ASAP), and the block
    schedule keeps DMA slightly ahead of the PE throughout. The last
    block is small so the serialized relu->L2->bias->store tail is short.
  * Engine layout: DVE does relu+bias epilogues, gpsimd/SWDGE stores
    per-block outputs so stores never head-of-line-block the x stream;
    the final store goes on the (by then idle) sync ring.
"""

from contextlib import ExitStack

import numpy as np

B = 65536
H = W = 28
K = 3
CH = CW = 26
FEAT = H * W          # 784
HID = 128
OUT = 10
NCORES = 8
BC = B // NCORES      # 8192 rows per core

KC = 112              # contraction-chunk partition size
KCH = 7               # chunks: 7 * 112 = 784
NT = 512              # max batch rows per compute tile (one PSUM bank fp32)
NWARM = 12            # 512-col dummy matmuls bridging preamble -> block 0
                      # (block 0 can't land before ~13us: first-DMA spin-up
                      # is ~4-5us; a post-warmup PE gap costs a 10us HALF-clock
                      # window, so over-provision)
X8SCALE = 2.0         # x is quantized as e3m4(2*x); 0.5 folded into w1

# variant "mixN": N chunks of x shipped as fp8-e3m4, 7-N as fp16.
# "f16" == mix0 (safe, ~5e-4 err), "f8" == mix7 (~1.65e-2 err).
VARIANT = "f8"

_NC_CACHE = {}


def _n8(variant):
    if variant == "f16":
        return 0
    if variant == "f8":
        return KCH
    if variant.startswith("mix"):
        n = int(variant[3:])
        assert 0 <= n <= KCH
        return n
    raise ValueError(variant)


def _blocks(bc):
    if bc == 8192:
        # small blocks while the PE is cold (1.2GHz), bigger once hot; the
        # tiny first block lands ASAP so the real stream starts early, the
        # tiny last block keeps the serialized epilogue tail short
        blocks = [256, 256, 512, 768, 1024, 1024, 1280, 1280, 1280, 384, 128]
    else:
        step = min(1024, bc)
        blocks = [min(step, bc - o) for o in range(0, bc, step)]
    assert sum(blocks) == bc
    return blocks


def _queues(blocks, wid):
    """Greedy queue assignment (0=sync, 1=ACT) balancing cumulative bytes.
    Sync starts with w1 (~200KB) on it, ACT with w2p (~8KB)."""
    load = [KC * KCH * HID * 2, 32 * HID * 2]
    qs = []
    for bi, xb in enumerate(blocks):
        if bi < 2:
            q = bi  # b0 leads the sync ring, b1 leads the ACT ring
        else:
            q = 0 if load[0] <= load[1] else 1
        qs.append(q)
        load[q] += KC * wid * xb
    return qs


def _tiles(xb):
    out, t0 = [], 0
    while t0 < xb:
        nt = min(NT, xb - t0)
        out.append((t0, nt))
        t0 += nt
    return out


def _build_nc(bc, variant):
    from concourse import bacc
    import concourse.mybir as mybir
    import concourse.tile as tile

    f32 = mybir.dt.float32
    f16 = mybir.dt.float16
    f8 = mybir.dt.float8e3
    u8 = mybir.dt.uint8
    n8 = _n8(variant)
    # chunk c dtype: first n8 chunks fp8, rest fp16
    csz = [1 if c < n8 else 2 for c in range(KCH)]
    cdt = [f8 if c < n8 else f16 for c in range(KCH)]
    coff = [sum(csz[:c]) for c in range(KCH)]  # byte offset factor per chunk
    wid = sum(csz)                             # bytes per column, all chunks
    blocks = _blocks(bc)
    queues = _queues(blocks, wid)

    nc = bacc.Bacc(
        "TRN2",
        target_bir_lowering=False,
        debug=False,
        enable_asserts=False,
        num_devices=NCORES,
    )
    # Per-queue dram tensor; per partition, that queue's blocks in order,
    # each block packed [chunk, col] so a block load is one contiguous run
    # of wid*xb bytes per partition.
    # NOTE: partition dim must stay 112 — a 113-partition DMA defeats the
    # HWDGE descriptor spray and funnels the whole queue through ONE SDMA
    # engine (measured: 8x bandwidth collapse).
    qbytes = [sum(wid * xb for xb, q in zip(blocks, queues) if q == i) for i in (0, 1)]
    xQ0 = nc.dram_tensor("xQ0", [KC, qbytes[0]], u8, kind="ExternalInput").ap()
    xQ1 = nc.dram_tensor("xQ1", [KC, qbytes[1]], u8, kind="ExternalInput").ap()
    w1t = nc.dram_tensor("w1t", [KC, KCH, HID], f16, kind="ExternalInput").ap()
    # w2, b2 and b1 packed as one [32, HID] tensor (rows 0-9 = w2, row 10 =
    # b2, row 11 = b1): 32 fat descriptors instead of 266 4-20B ones (each
    # tiny descriptor is a full HBM round trip), transposed on the DVE
    w2p = nc.dram_tensor("w2p", [32, HID], f16, kind="ExternalInput").ap()
    outT = nc.dram_tensor("outT", [OUT, bc], f32, kind="ExternalOutput").ap()

    with ExitStack() as ctx:
        tc = ctx.enter_context(tile.TileContext(nc))
        wpool = ctx.enter_context(tc.tile_pool(name="w", bufs=1))
        xpool = ctx.enter_context(tc.tile_pool(name="x", bufs=3))  # >= max same-size blocks
        hpool = ctx.enter_context(tc.tile_pool(name="h", bufs=3))
        opool = ctx.enter_context(tc.tile_pool(name="o", bufs=4))
        p1pool = ctx.enter_context(tc.tile_pool(name="p1", bufs=5, space="PSUM"))
        p2pool = ctx.enter_context(tc.tile_pool(name="p2", bufs=3, space="PSUM"))

        qeng = [nc.sync, nc.scalar]
        qoff = [0, 0]      # running byte offset within each queue tensor
        qt = [xQ0, xQ1]
        xs = [None] * len(blocks)
        xoff = [None] * len(blocks)

        def issue_x(bi):
            xb = blocks[bi]
            q = queues[bi]
            # per-size tags with enough bufs that every block is resident:
            # no buffer reuse, so no DMA ever waits on a stale reader
            xs[bi] = xpool.tile(
                [KC, wid * xb], u8, tag=f"xs_{xb}", name=f"xs_{bi}"
            )
            qeng[q].dma_start(xs[bi][:], qt[q][:, qoff[q] : qoff[q] + wid * xb])
            qoff[q] += wid * xb

        # w1 chunk 0 is its own small DMA so the very first matmul doesn't
        # wait on all 7 chunks; x block 0 goes right behind it, then the
        # w1 bulk. On the ACT ring w2p (tiny) leads its first x block.
        w1s = wpool.tile([KC, KCH, HID], f16)
        w2ps = wpool.tile([32, HID], f16)
        issue_x(0)
        issue_x(1)
        nc.sync.dma_start(w1s[:, 0:1, :], w1t[:, 0:1, :])
        nc.scalar.dma_start(w2ps[:], w2p[:])
        nc.sync.dma_start(w1s[:, 1:KCH, :], w1t[:, 1:KCH, :])
        # issue every remaining block DMA upfront: the whole x shard fits
        # SBUF, so each block gets its own buffer and the two queues just
        # stream back-to-back with nothing interleaved into their rings
        for bi in range(2, len(blocks)):
            issue_x(bi)

        w2sT = wpool.tile([HID, 32], f16)
        for j in range(HID // 32):
            # DVE transpose flips one 32x32 block; stitch the full transpose
            nc.vector.transpose(
                w2sT[32 * j : 32 * (j + 1), 0:32], w2ps[0:32, 32 * j : 32 * (j + 1)]
            )
        w2l = w2sT[:, 0:OUT]        # layer-2 lhsT [128, 10]
        bvec = wpool.tile([HID, 2], f32)  # tensor_scalar wants f32 scalars
        nc.vector.tensor_copy(bvec[:], w2sT[:, OUT : OUT + 2])
        b2s = bvec[0:OUT, 0:1]      # b2 as per-partition scalar [10, 1]
        b1s = bvec[:, 1:2]          # b1 as per-partition scalar [128, 1]

        # PE warm-up: the HAM clock gate holds the PE at 1.2 GHz until it
        # has seen ~5us of sustained activity, and a PE stall resets the
        # ramp. Burn dummy matmuls on scratch during block 0's DMA so the
        # PE never idles between the framework preamble and the real
        # stream. Garbage operands are fine: start=True overwrites PSUM,
        # and the first real matmul overwrites it again.
        warmW = wpool.tile([KC, HID], f16)
        warmM = wpool.tile([KC, NT], f16)
        nc.gpsimd.memset(warmW[:], 0.0)   # two engines so the first warm
        nc.vector.memset(warmM[:], 0.0)   # LDWEIGHTS isn't gated on one big memset
        pwarm = p1pool.tile([HID, NT], f32, tag="p1", name="p1_warm")
        for i in range(NWARM):
            nc.tensor.matmul(
                pwarm[:], warmW[:], warmM[:],
                start=True, stop=True, skip_group_check=True,
            )

        # global tile list: (blk_idx, xb, t0, nt, first_of_block)
        gtiles = []
        for bi, xb in enumerate(blocks):
            for ti, (t0, nt) in enumerate(_tiles(xb)):
                gtiles.append((bi, xb, t0, nt, ti == 0))

        os_ = [None] * len(blocks)
        done_tiles = [0] * len(blocks)  # epilogues emitted per block
        ntiles_of = [len(_tiles(xb)) for xb in blocks]
        # software pipeline: the L2 matmul of tile j-2 is emitted while the
        # PE chews on tile j's layer-1, so the DVE relu (emitted right after
        # tile j-2's layer-1) has two full tiles of slack before the PE
        # needs its output
        pend = []  # [(p1 tile, bi, t0, nt), ...]

        add = mybir.AluOpType.add
        mx = mybir.AluOpType.max

        def flush(k):
            # drain k pending tiles as a batch: both relus first (DVE),
            # then both L2 matmuls back-to-back (one w2l->w1c0 PE weight
            # transition per batch instead of per tile), then the bias
            # epilogues on the otherwise-idle gpsimd engine
            batch = [pend.pop(0) for _ in range(k)]
            hss = []
            for p1, bi, t0, nt in batch:
                hs = hpool.tile([HID, nt], f16, tag="hs", name=f"hs_{bi}_{t0}")
                nc.vector.tensor_scalar(hs[:], p1[:], b1s, 0.0, add, mx)
                hss.append(hs)
            p2s = []
            for (p1, bi, t0, nt), hs in zip(batch, hss):
                p2 = p2pool.tile([OUT, nt], f32, tag="p2", name=f"p2_{bi}_{t0}")
                nc.tensor.matmul(p2[:], w2l, hs[:], start=True, stop=True)
                p2s.append(p2)
            for (p1, bi, t0, nt), p2 in zip(batch, p2s):
                # PSUM is only readable by DVE/ACT, and epilogues on the
                # ACT ring would gate its DMA issues on compute progress
                # (measured: starves the x stream) — so DVE it is, with
                # both relus emitted ahead of both biases
                nc.vector.tensor_scalar_add(os_[bi][:, t0 : t0 + nt], p2[:], b2s)
                done_tiles[bi] += 1
                # store a fully finished block via SWDGE (never blocks x loads)
                if done_tiles[bi] == ntiles_of[bi] and bi < len(blocks) - 1:
                    boff = sum(blocks[:bi])
                    nc.gpsimd.dma_start(outT[:, boff : boff + blocks[bi]], os_[bi][:])
                    os_[bi] = None

        for bi, xb, t0, nt, first in gtiles:
            if first:
                os_[bi] = opool.tile([OUT, xb], f32, tag=f"os_{xb}", name=f"os_{bi}")

            # emit the epilogues for the tiles the DVE should do next,
            # BEFORE this tile's matmuls, so the PE keeps 2+ tiles of slack
            if len(pend) >= 4:
                flush(2)

            p1 = p1pool.tile([HID, nt], f32, tag="p1", name=f"p1_{bi}_{t0}")
            for c in range(KCH):
                base = coff[c] * xb
                if csz[c] == 1:
                    rhs = xs[bi][:, base + t0 : base + t0 + nt].bitcast(f8)
                else:
                    rhs = xs[bi][:, base + 2 * t0 : base + 2 * (t0 + nt)].bitcast(f16)
                nc.tensor.matmul(
                    p1[:],
                    w1s[:, c, :],
                    rhs,
                    start=(c == 0),
                    stop=(c == KCH - 1),
                )
            pend.append((p1, bi, t0, nt))

        while pend:
            flush(min(2, len(pend)))
        off = 0
        for bi, xb in enumerate(blocks):
            if os_[bi] is not None:
                # the final stores go on the (now idle) sync ring: HWDGE
                # latency on the critical tail
                nc.sync.dma_start(outT[:, off : off + xb], os_[bi][:])
                os_[bi] = None
            off += xb

    nc.compile()
    return nc


def get_nc(bc=BC, variant=VARIANT):
    key = (bc, variant)
    if key not in _NC_CACHE:
        _NC_CACHE[key] = _build_nc(bc, variant)
    return _NC_CACHE[key]


def _host_prep(x, conv_w, w1, b1, w2, b2, variant):
    """Fold conv into layer-1 weights and lay out per-core device inputs."""
    import ml_dtypes

    n8 = _n8(variant)
    csz = [1 if c < n8 else 2 for c in range(KCH)]
    wid = sum(csz)

    x = np.asarray(x, dtype=np.float32)
    conv_w = np.asarray(conv_w, dtype=np.float32)
    w1 = np.asarray(w1, dtype=np.float32)
    b1 = np.asarray(b1, dtype=np.float32)
    w2 = np.asarray(w2, dtype=np.float32)
    b2 = np.asarray(b2, dtype=np.float32)

    w1_img = w1.reshape(HID, CH, CW)
    w1eff = np.zeros((HID, H, W), dtype=np.float32)
    for di in range(K):
        for dj in range(K):
            w1eff[:, di : di + CH, dj : dj + CW] += conv_w[di, dj] * w1_img
    w1eff = w1eff.reshape(HID, FEAT)

    # [784,128] -> [7,112,128] -> [112,7,128]; fp8 chunks carry the folded
    # 1/X8SCALE so the device dequant is free
    w1full = w1eff.T.reshape(KCH, KC, HID).copy()
    w1full[:n8] *= 1.0 / X8SCALE
    w1t_host = np.ascontiguousarray(w1full.transpose(1, 0, 2)).astype(np.float16)
    # rows 0-9: w2; row 10: b2; row 11: b1 (device reads them back out of
    # the DVE-transposed tile as per-partition scalar columns)
    w2p_host = np.zeros((32, HID), dtype=np.float16)
    w2p_host[:OUT] = w2.astype(np.float16)
    w2p_host[OUT, :OUT] = b2.astype(np.float16)
    w2p_host[OUT + 1] = b1.astype(np.float16)

    blocks = _blocks(BC)
    queues = _queues(blocks, wid)
    in_maps = []
    for c in range(NCORES):
        shardT = x[c * BC : (c + 1) * BC].T  # [784, BC] view
        xr = np.ascontiguousarray(shardT).reshape(KCH, KC, BC)
        # per-chunk byte rows [KC, csz*BC]
        rows = []
        for ci in range(KCH):
            if csz[ci] == 1:
                q = (xr[ci] * X8SCALE).astype(ml_dtypes.float8_e3m4)
                rows.append(q.view(np.uint8))
            else:
                q = np.ascontiguousarray(xr[ci].astype(np.float16))
                rows.append(q.view(np.uint8).reshape(KC, 2 * BC))
        qhost = [np.empty((KC, 0), np.uint8), np.empty((KC, 0), np.uint8)]
        parts = [[], []]
        off = 0
        for xb, qi in zip(blocks, queues):
            for ci in range(KCH):
                w = csz[ci]
                parts[qi].append(rows[ci][:, w * off : w * (off + xb)])
            off += xb
        qhost = [
            np.concatenate(p, axis=1) if p else np.empty((KC, 0), np.uint8)
            for p in parts
        ]
        im = {
            "w1t": w1t_host,
            "w2p": w2p_host,
            "xQ0": np.ascontiguousarray(qhost[0]),
            "xQ1": np.ascontiguousarray(qhost[1]),
        }
        in_maps.append(im)
    return in_maps


def run(x, conv_w, w1, b1, w2, b2, trace=False, variant=VARIANT):
    from concourse.bass_utils import run_bass_kernel_spmd

    in_maps = _host_prep(x, conv_w, w1, b1, w2, b2, variant)
    nc = get_nc(BC, variant)
    res = run_bass_kernel_spmd(nc, in_maps, list(range(NCORES)), trace=trace)
    outT = np.concatenate([r["outT"] for r in res.results], axis=1)  # [10, B]
    return np.ascontiguousarray(outT.T), res


def kernel(x, conv_w, w1, b1, w2, b2):
    out, _ = run(x, conv_w, w1, b1, w2, b2)
    return out
